# revision 4
# baseline (speedup 1.0000x reference)
"""Involution-style per-pixel depthwise 3x3 conv on 8 trn2 NeuronCores.

out[n,c,h,w] = sum_{k=0..8} w[n,c,k,h,w] * x_pad[n,c,h+k//3,w+k%3]  (pad=1)

Sharding: pure data parallel over N=8 -> one sample per core.
Per core: channels C=128 = SBUF partition dim; free dim = H*W pixels.
Padding is handled with per-tap valid sub-rectangles (border output pixels
accumulate fewer taps, which is exactly zero-padding semantics).

Compute split per 32-row stripe:
  - mid taps (3,4,5): DVE; center tap 4 initializes the accumulator.
  - top taps (0,1,2): GPSIMD, accumulated into their own partial (p1).
  - bot taps (6,7,8): DVE chain (p7), with tap-8's product on GPSIMD.
  - merge: acc += p1; acc += p7 on DVE.
Products are computed in-place in the weight slab tile (no tmp tiles).
"""

import numpy as np

import concourse.bass as bass
import concourse.mybir as mybir
from concourse.bass_utils import run_bass_kernel_spmd
from concourse.tile import TileContext

N_CORES = 8
C, H, W = 128, 96, 96
HW = H * W
KW = 3

R = 32                # stripe rows
NSTR = H // R         # 3 stripes
SL = R * W            # elems per stripe per partition

F32 = mybir.dt.float32

# taps whose product (in-place in the slab) runs on GPSIMD
GPSIMD_MULTS = {0, 1, 2, 8}


def _rect(k: int, r0: int, r1: int):
    """Valid out-pixel rectangle for tap k intersected with stripe rows [r0, r1).

    Returns (h0, h1, w0, w1) in global out coords, or None if empty.
    """
    di, dj = k // KW - 1, k % KW - 1
    h0 = max(r0, max(0, -di))
    h1 = min(r1, H - max(0, di))
    w0 = max(0, -dj)
    w1 = W - max(0, dj)
    if h0 >= h1 or w0 >= w1:
        return None
    return h0, h1, w0, w1


def _build() -> bass.Bass:
    nc = bass.Bass()
    x_d = nc.dram_tensor("x", [C, HW], F32, kind="ExternalInput")
    w_d = nc.dram_tensor("w", [C * KW * KW, HW], F32, kind="ExternalInput")
    o_d = nc.dram_tensor("out", [C, HW], F32, kind="ExternalOutput")

    # [c, k, h*w] view of the weights in HBM
    w_v = w_d[:].rearrange("(c k) m -> c k m", k=KW * KW)

    with TileContext(nc) as tc:
        with (
            tc.tile_pool(name="px", bufs=1) as px,
            tc.tile_pool(name="pw", bufs=3) as pw,
            tc.tile_pool(name="pa", bufs=2) as pa,
        ):
            x_t = px.tile([C, HW], F32)
            nc.sync.dma_start(out=x_t[:, :], in_=x_d[:, :])
            x_r = x_t[:, :].rearrange("p (h w) -> p h w", w=W)

            for s in range(NSTR):
                r0, r1 = s * R, (s + 1) * R

                acc = pa.tile([C, SL], F32, tag="acc")
                acc_r = acc.rearrange("p (h w) -> p h w", w=W)

                # one slab per row-group; groups ordered mid, top, bot
                slabs = {}
                for gname, k0 in (("mid", 3), ("top", 0), ("bot", 6)):
                    slab = pw.tile([C, KW, SL], F32, tag="w", name=f"w_{gname}_{s}")
                    nc.sync.dma_start(
                        out=slab, in_=w_v[:, k0 : k0 + KW, r0 * W : r1 * W]
                    )
                    slabs[gname] = slab.rearrange("p k (h w) -> p k h w", w=W)

                def wview(k, rect):
                    g, k0 = ("mid", 3) if 3 <= k < 6 else (("top", 0) if k < 3 else ("bot", 6))
                    h0, h1, w0, w1 = rect
                    return slabs[g][:, k - k0, h0 - r0 : h1 - r0, w0:w1]

                def xview(k, rect):
                    di, dj = k // KW - 1, k % KW - 1
                    h0, h1, w0, w1 = rect
                    return x_r[:, h0 + di : h1 + di, w0 + dj : w1 + dj]

                def accview(rect):
                    h0, h1, w0, w1 = rect
                    return acc_r[:, h0 - r0 : h1 - r0, w0:w1]

                def mult(k, out_ap=None):
                    """product of tap k, in-place in the slab unless out_ap given"""
                    rect = _rect(k, r0, r1)
                    assert rect is not None
                    wv = wview(k, rect)
                    eng = nc.gpsimd if (k in GPSIMD_MULTS and out_ap is None) else nc.vector
                    eng.tensor_mul(out=wv if out_ap is None else out_ap, in0=wv, in1=xview(k, rect))
                    return rect

                # --- mid group on DVE: acc = p4; acc += p3; acc += p5 ---
                rect4 = _rect(4, r0, r1)
                nc.vector.tensor_mul(
                    out=accview(rect4), in0=wview(4, rect4), in1=xview(4, rect4)
                )
                for k in (3, 5):
                    rect = mult(k)
                    nc.vector.tensor_add(
                        out=accview(rect), in0=accview(rect), in1=wview(k, rect)
                    )

                # --- top group on GPSIMD: p1 = w1*x; p1 += p0; p1 += p2 ---
                rect1 = mult(1)
                for k in (0, 2):
                    rect = mult(k)
                    # p1 and pk are indexed by the same out coords
                    h0, h1, w0, w1 = rect
                    p1_at = slabs["top"][:, 1, h0 - r0 : h1 - r0, w0:w1]
                    nc.gpsimd.tensor_add(out=p1_at, in0=p1_at, in1=wview(k, rect))

                # --- bot group: p7 = w7*x (DVE); p7 += p6 (DVE); p7 += p8 (gpsimd mult) ---
                rect7 = _rect(7, r0, r1)
                if rect7 is not None:
                    nc.vector.tensor_mul(
                        out=wview(7, rect7), in0=wview(7, rect7), in1=xview(7, rect7)
                    )
                    for k in (6, 8):
                        rect = mult(k)
                        h0, h1, w0, w1 = rect
                        p7_at = slabs["bot"][:, 1, h0 - r0 : h1 - r0, w0:w1]
                        nc.vector.tensor_add(out=p7_at, in0=p7_at, in1=wview(k, rect))

                # --- merges on DVE ---
                h0, h1, w0, w1 = rect1
                nc.vector.tensor_add(
                    out=acc_r[:, h0 - r0 : h1 - r0, w0:w1],
                    in0=acc_r[:, h0 - r0 : h1 - r0, w0:w1],
                    in1=slabs["top"][:, 1, h0 - r0 : h1 - r0, w0:w1],
                )
                if rect7 is not None:
                    h0, h1, w0, w1 = rect7
                    nc.vector.tensor_add(
                        out=acc_r[:, h0 - r0 : h1 - r0, w0:w1],
                        in0=acc_r[:, h0 - r0 : h1 - r0, w0:w1],
                        in1=slabs["bot"][:, 1, h0 - r0 : h1 - r0, w0:w1],
                    )

                nc.sync.dma_start(out=o_d[:, r0 * W : r1 * W], in_=acc[:, :])

    return nc


def _split_excess_waits(nc: bass.Bass) -> None:
    """TPB engine instructions carry exactly ONE sync-wait slot; walrus
    refuses instructions with more ("Too many sync wait commands"). Tile's
    sem assignment can emit several waits on one instruction. Split the
    extras onto same-engine NOPs inserted immediately before the
    instruction — the engine sequencer executes them in order, so all
    waits are still satisfied before the instruction runs."""
    import bass_rust

    f = nc.m.functions[0]

    def make_nop(engine):
        ins = nc.engines[engine].nop().ins
        # nop() appends to the currently-open bb; detach it from there
        for bb in f.blocks:
            il = bb.instructions
            for j in range(len(il) - 1, -1, -1):
                if il[j].name == ins.name:
                    del il[j]
                    return ins
        raise AssertionError("freshly created nop not found in any block")

    for bb in f.blocks:
        il = bb.instructions
        i = 0
        while i < len(il):
            ins = il[i]
            si = ins.sync_info
            waits = list(si.on_wait) if si and si.on_wait else []
            if len(waits) > 1:
                updates = list(si.on_update) if si.on_update else []
                ins.sync_info = bass_rust.SyncInfo(
                    on_wait=[waits[-1]], on_update=updates
                )
                for k, w in enumerate(waits[:-1]):
                    nop = make_nop(ins.engine)
                    nop.sync_info = bass_rust.SyncInfo(on_wait=[w], on_update=[])
                    il.insert(i + k, nop)
                i += len(waits) - 1
            i += 1


_NC_CACHE = None


def _get_nc():
    global _NC_CACHE
    if _NC_CACHE is None:
        nc = _build()
        _split_excess_waits(nc)
        _NC_CACHE = nc
    return _NC_CACHE


_RUNNER = None


def _get_runner():
    """Jit the SPMD executable once; repeated kernel() calls reuse it.

    Mirrors concourse.bass2jax.run_bass_via_pjrt's multi-core branch but
    caches the jitted callable (run_bass_via_pjrt builds a fresh closure
    per call, forcing an XLA recompile every time)."""
    global _RUNNER
    if _RUNNER is not None:
        return _RUNNER

    import jax
    from jax.experimental.shard_map import shard_map
    from jax.sharding import Mesh, PartitionSpec

    import concourse.mybir as _mybir
    from concourse import bass2jax

    bass2jax.install_neuronx_cc_hook()
    nc = _get_nc()

    partition_name = (
        nc.partition_id_tensor.name if nc.partition_id_tensor else None
    )
    in_names, out_names, out_avals = [], [], []
    for alloc in nc.m.functions[0].allocations:
        if not isinstance(alloc, _mybir.MemoryLocationSet):
            continue
        name = alloc.memorylocations[0].name
        if alloc.kind == "ExternalInput":
            if name != partition_name:
                in_names.append(name)
        elif alloc.kind == "ExternalOutput":
            out_names.append(name)
            out_avals.append(
                jax.core.ShapedArray(
                    tuple(alloc.tensor_shape), _mybir.dt.np(alloc.dtype)
                )
            )
    n_params = len(in_names)
    n_outs = len(out_names)
    all_in_names = tuple(in_names + out_names)
    if partition_name is not None:
        all_in_names = all_in_names + (partition_name,)
    donate = tuple(range(n_params, n_params + n_outs))

    def _body(*args):
        operands = list(args)
        if partition_name is not None:
            operands.append(bass2jax.partition_id_tensor())
        outs = bass2jax._bass_exec_p.bind(
            *operands,
            out_avals=tuple(out_avals),
            in_names=all_in_names,
            out_names=tuple(out_names),
            lowering_input_output_aliases=(),
            sim_require_finite=True,
            sim_require_nnan=True,
            nc=nc,
        )
        return tuple(outs)

    devices = jax.devices()[:N_CORES]
    mesh = Mesh(np.asarray(devices), ("core",))
    sharded = jax.jit(
        shard_map(
            _body,
            mesh=mesh,
            in_specs=(PartitionSpec("core"),) * (n_params + n_outs),
            out_specs=(PartitionSpec("core"),) * n_outs,
            check_rep=False,
        ),
        donate_argnums=donate,
        keep_unused=True,
    )

    def runner(concat_inputs):
        zeros = [
            np.zeros((N_CORES * a.shape[0], *a.shape[1:]), a.dtype) for a in out_avals
        ]
        outs = sharded(*concat_inputs, *zeros)
        return [np.asarray(o) for o in outs]

    _RUNNER = (runner, in_names, out_names, out_avals)
    return _RUNNER


def prep_inputs(x, conv_weights):
    """Reshape full inputs into the concatenated per-core layout."""
    x = np.ascontiguousarray(np.asarray(x, dtype=np.float32))
    w = np.ascontiguousarray(np.asarray(conv_weights, dtype=np.float32))
    assert x.shape == (N_CORES, C, H, W), x.shape
    assert w.shape == (N_CORES, C * KW * KW, H, W), w.shape
    by_name = {
        "x": x.reshape(N_CORES * C, HW),
        "w": w.reshape(N_CORES * C * KW * KW, HW),
    }
    _, in_names, _, _ = _get_runner()
    return [by_name[n] for n in in_names]


def execute(concat_inputs):
    runner, _, out_names, out_avals = _get_runner()
    outs = runner(concat_inputs)
    i = out_names.index("out")
    return outs[i].reshape(N_CORES, C, H, W)


def kernel(x, conv_weights):
    return execute(prep_inputs(x, conv_weights))


def run(x, conv_weights, **spmd_kwargs):
    """Legacy full-path entry via run_bass_kernel_spmd (no jit caching)."""
    x = np.ascontiguousarray(np.asarray(x, dtype=np.float32))
    w = np.ascontiguousarray(np.asarray(conv_weights, dtype=np.float32))
    n = x.shape[0]
    nc = _get_nc()
    in_maps = [
        {"x": x[i].reshape(C, HW), "w": w[i].reshape(C * KW * KW, HW)}
        for i in range(n)
    ]
    br = run_bass_kernel_spmd(nc, in_maps, core_ids=list(range(n)), **spmd_kwargs)
    out = np.stack([r["out"].reshape(C, H, W) for r in br.results])
    return out, br
